# revision 1
# baseline (speedup 1.0000x reference)
"""DSS layer (LN -> long causal conv via overlap-save DFT matmuls -> +residual)
on 8 axon-tunneled TRN2 NeuronCores, written in Bass/Tile.

Wall-clock on this setup is dominated by the ~60 MB/s (up) / ~36 MB/s (down)
axon tunnel, so the design minimizes transferred bytes:
  host: LN + per-row int8 quantization of the normalized signal (upload int8),
        conv kernel K computed exactly and truncated at 513 taps (decay ~1e-10),
        gamma / D-residual (delta tap) / per-channel output scale folded into
        the kernel spectrum Kf; beta handled exactly by a host-side offset.
  device (per core = one (batch, L-half), 2048 own rows + 512 halo rows):
        dequant -> windowed rFFT-as-matmul (shared F), pointwise *Kf,
        inverse rFFT-as-matmul (shared G) -> uint8 quantize (round-to-nearest).
  download uint8, host dequant (s_d per channel) + beta offset.

Execution mirrors concourse.bass_utils.run_bass_kernel_spmd's axon redirect
(bass2jax custom call over PJRT shard_map), but with the jitted executable and
device-resident constants cached across calls.
"""
import hashlib
import threading
import numpy as np
import ml_dtypes

B, L, D, N = 4, 4096, 1024, 512
CH = 512            # output chunk per window
M = 1024            # DFT window (overlap-save)
KT = 513            # kernel taps kept (<= M - CH + 1): exact for decaying K
KF = M // 2 + 1     # 513 rfft bins
HALO = 512
OWN = L // 2        # 2048 rows per core
ROWS = OWN + HALO   # 2560
NCORE = 8
LN_EPS = 1e-5
QCLIP = 5.2
KPART = [(0, 128), (128, 128), (256, 128), (384, 128), (512, 1)]

_S = {}
_LOCK = threading.Lock()


# ---------------------------------------------------------------- device kernel
def _build_nc():
    import concourse.bacc as bacc
    import concourse.mybir as mybir
    import concourse.tile as tile

    dt = mybir.dt
    nc = bacc.Bacc("TRN2", target_bir_lowering=False, debug=False, num_devices=NCORE)
    uq_d = nc.dram_tensor("uq", [ROWS, D], dt.int8, kind="ExternalInput").ap()
    sr_d = nc.dram_tensor("srow", [ROWS], dt.float32, kind="ExternalInput").ap()
    kr_d = nc.dram_tensor("kr", [KF, D], dt.bfloat16, kind="ExternalInput").ap()
    ki_d = nc.dram_tensor("ki", [KF, D], dt.bfloat16, kind="ExternalInput").ap()
    fc_d = nc.dram_tensor("fc", [M, KF], dt.bfloat16, kind="ExternalInput").ap()
    fs_d = nc.dram_tensor("fs", [M, KF], dt.bfloat16, kind="ExternalInput").ap()
    gr_d = nc.dram_tensor("gr", [KF, CH], dt.bfloat16, kind="ExternalInput").ap()
    gi_d = nc.dram_tensor("gi", [KF, CH], dt.bfloat16, kind="ExternalInput").ap()
    yq_d = nc.dram_tensor("yq", [OWN, D], dt.uint8, kind="ExternalOutput").ap()

    with tile.TileContext(nc) as tc:
        with (
            tc.tile_pool(name="const", bufs=1) as constp,
            tc.tile_pool(name="stage", bufs=2) as stagep,
            tc.tile_pool(name="upool", bufs=9) as upool,
            tc.tile_pool(name="uv", bufs=2) as uvp,
            tc.tile_pool(name="work", bufs=2) as workp,
            tc.tile_pool(name="psum", bufs=4, space="PSUM") as psump,
            tc.tile_pool(name="psumi", bufs=2, space="PSUM") as psumip,
        ):
            def widen(dram_ap, rows, cols, tagn):
                st = stagep.tile([rows, cols], dt.bfloat16, tag="stage")
                nc.sync.dma_start(st[:], dram_ap)
                ft = constp.tile([rows, cols], dt.float32, tag=tagn)
                nc.vector.tensor_copy(ft[:], st[:])
                return ft

            fc_t = [widen(fc_d[i * 128:(i + 1) * 128, :], 128, KF, f"fc{i}") for i in range(8)]
            fs_t = [widen(fs_d[i * 128:(i + 1) * 128, :], 128, KF, f"fs{i}") for i in range(8)]
            gr_t = [widen(gr_d[o:o + w, :], w, CH, f"gr{i}") for i, (o, w) in enumerate(KPART)]
            gi_t = [widen(gi_d[o:o + w, :], w, CH, f"gi{i}") for i, (o, w) in enumerate(KPART)]

            # Kf stays bf16 in SBUF (read by DVE pointwise; halves footprint)
            def load_bf(dram_ap, rows, cols, tagn):
                t = constp.tile([rows, cols], dt.bfloat16, tag=tagn)
                nc.sync.dma_start(t[:], dram_ap)
                return t

            kr_t = [load_bf(kr_d[o:o + w, :], w, D, f"kr{i}") for i, (o, w) in enumerate(KPART)]
            ki_t = [load_bf(ki_d[o:o + w, :], w, D, f"ki{i}") for i, (o, w) in enumerate(KPART)]

            nT = ROWS // 128  # 20
            sr_raw = constp.tile([128, nT], dt.float32, tag="sr_raw")
            nc.sync.dma_start(sr_raw[:], sr_d.rearrange("(n p) -> p n", p=128))
            # staged via same-engine copy so dequant TensorScalarPtr needs no waits
            sr_sb = constp.tile([128, nT], dt.float32, tag="sr_sb")
            nc.vector.tensor_copy(sr_sb[:], sr_raw[:])

            for c in range(L // 2 // CH):  # 4 windows
                u_t = []
                for j in range(8):
                    ti = c * 4 + j
                    stq = stagep.tile([128, D], dt.int8, tag="uqstage")
                    nc.sync.dma_start(stq[:], uq_d[ti * 128:(ti + 1) * 128, :])
                    uf = upool.tile([128, D], dt.float32, tag="u")
                    nc.vector.tensor_copy(uf[:], stq[:])
                    nc.vector.tensor_scalar_mul(uf[:], uf[:], sr_sb[:, ti:ti + 1])
                    u_t.append(uf)
                for dh in range(2):
                    dsl = slice(dh * 512, dh * 512 + 512)
                    Vr, Vi = [], []
                    for it, (ko, kw) in enumerate(KPART):
                        sb_ri = []
                        for nm, fT in (("ur", fc_t), ("ui", fs_t)):
                            ps = psump.tile([kw, 512], dt.float32, tag="psf")
                            for si in range(8):
                                nc.tensor.matmul(
                                    ps[:], fT[si][:, ko:ko + kw], u_t[si][:, dsl],
                                    start=(si == 0), stop=(si == 7),
                                )
                            sb = uvp.tile([kw, 512], dt.float32, tag=nm)
                            nc.scalar.copy(sb[:], ps[:])
                            sb_ri.append(sb)
                        ur, ui = sb_ri
                        krs, kis = kr_t[it][:kw, dsl], ki_t[it][:kw, dsl]
                        t1 = workp.tile([kw, 512], dt.float32, tag="t1")
                        t2 = workp.tile([kw, 512], dt.float32, tag="t2")
                        nc.vector.tensor_mul(t1[:], ur[:], krs)
                        nc.vector.tensor_mul(t2[:], ui[:], kis)
                        vr = uvp.tile([kw, 512], dt.float32, tag=f"vr{it}")
                        nc.vector.tensor_sub(vr[:], t1[:], t2[:])
                        t3 = workp.tile([kw, 512], dt.float32, tag="t3")
                        t4 = workp.tile([kw, 512], dt.float32, tag="t4")
                        nc.vector.tensor_mul(t3[:], ur[:], kis)
                        nc.vector.tensor_mul(t4[:], ui[:], krs)
                        vi = uvp.tile([kw, 512], dt.float32, tag=f"vi{it}")
                        nc.vector.tensor_add(vi[:], t3[:], t4[:])
                        Vr.append(vr)
                        Vi.append(vi)
                    for tt in range(4):
                        ps = psumip.tile([128, 512], dt.float32, tag="psi")
                        mm = 0
                        for gT, V in ((gr_t, Vr), (gi_t, Vi)):
                            for it, (ko, kw) in enumerate(KPART):
                                nc.tensor.matmul(
                                    ps[:], gT[it][:kw, tt * 128:(tt + 1) * 128], V[it][:],
                                    start=(mm == 0), stop=(mm == 9),
                                )
                                mm += 1
                        yf = workp.tile([128, 512], dt.float32, tag="yf")
                        nc.vector.tensor_scalar_add(yf[:], ps[:], 128.0)
                        nc.vector.tensor_scalar_max(yf[:], yf[:], 1.0)
                        nc.vector.tensor_scalar_min(yf[:], yf[:], 255.0)
                        yq_t = workp.tile([128, 512], dt.uint8, tag="yqt")
                        nc.vector.tensor_copy(yq_t[:], yf[:])
                        nc.sync.dma_start(
                            yq_d[c * CH + tt * 128: c * CH + (tt + 1) * 128, dsl], yq_t[:]
                        )
    nc.finalize()
    return nc


# ---------------------------------------------------------------- runner
def _make_runner(nc):
    import jax
    from jax.sharding import Mesh, PartitionSpec
    from jax.experimental.shard_map import shard_map
    import concourse.mybir as mybir
    from concourse.bass2jax import install_neuronx_cc_hook, _bass_exec_p, partition_id_tensor

    install_neuronx_cc_hook()
    in_names, out_names, out_avals, zero_outs = [], [], [], []
    partition_name = nc.partition_id_tensor.name if nc.partition_id_tensor else None
    for alloc in nc.m.functions[0].allocations:
        if not isinstance(alloc, mybir.MemoryLocationSet):
            continue
        name = alloc.memorylocations[0].name
        if alloc.kind == "ExternalInput":
            if name != partition_name:
                in_names.append(name)
        elif alloc.kind == "ExternalOutput":
            out_names.append(name)
            shape = tuple(alloc.tensor_shape)
            dtype = mybir.dt.np(alloc.dtype)
            out_avals.append(jax.core.ShapedArray(shape, dtype))
            zero_outs.append(np.zeros(shape, dtype))
    n_params = len(in_names)
    all_names = in_names + out_names
    if partition_name is not None:
        all_names.append(partition_name)

    def _body(*args):
        operands = list(args)
        if partition_name is not None:
            operands.append(partition_id_tensor())
        outs = _bass_exec_p.bind(
            *operands,
            out_avals=tuple(out_avals),
            in_names=tuple(all_names),
            out_names=tuple(out_names),
            lowering_input_output_aliases=(),
            sim_require_finite=True,
            sim_require_nnan=True,
            nc=nc,
        )
        return tuple(outs)

    devices = jax.devices()[:NCORE]
    mesh = Mesh(np.asarray(devices), ("core",))
    n_outs = len(out_names)
    sharded = jax.jit(
        shard_map(
            _body, mesh=mesh,
            in_specs=(PartitionSpec("core"),) * (n_params + n_outs),
            out_specs=(PartitionSpec("core"),) * n_outs,
            check_rep=False,
        ),
        keep_unused=True,
    )
    return sharded, in_names, out_names, zero_outs, mesh


def _dft_consts():
    t = np.arange(M)
    k = np.arange(KF)
    ang = 2.0 * np.pi / M * np.outer(t, k)
    fc = np.cos(ang)
    fs = -np.sin(ang)
    w_k = np.where((k == 0) | (k == M // 2), 1.0, 2.0) / M
    angi = 2.0 * np.pi / M * np.outer(k, np.arange(CH, M))
    gr = w_k[:, None] * np.cos(angi)
    gi = -w_k[:, None] * np.sin(angi)
    bf = ml_dtypes.bfloat16
    return (fc.astype(bf), fs.astype(bf), gr.astype(bf), gi.astype(bf))


def _init():
    import jax
    from jax.sharding import NamedSharding, PartitionSpec

    nc = _build_nc()
    sharded, in_names, out_names, zero_outs, mesh = _make_runner(nc)
    assert in_names == ["uq", "srow", "kr", "ki", "fc", "fs", "gr", "gi"], in_names
    repl = NamedSharding(mesh, PartitionSpec("core"))
    fc, fs, gr, gi = _dft_consts()
    tile8 = lambda a: jax.device_put(np.tile(a, (NCORE, 1)), repl)
    _S["fgdev"] = (tile8(fc), tile8(fs), tile8(gr), tile8(gi))
    zc = np.zeros((NCORE * OWN, D), np.uint8)
    _S["zeros"] = jax.device_put(zc, repl)
    _S["sharded"] = sharded
    _S["repl"] = repl
    _S["kcache"] = {}
    _S["ready"] = True


# ---------------------------------------------------------------- host math
def _host_precompute(Lr, Li, Cr, Ci, Dp, g, b):
    lam = -np.exp(Lr.astype(np.float64)) + 1j * np.exp(Li.astype(np.float64))
    Ct = (Cr.astype(np.float64) + 1j * Ci.astype(np.float64)) * (np.exp(lam) - 1.0) / lam
    tau = np.arange(KT)
    E = np.exp(lam[None, :] * tau[:, None])  # [KT, N]
    K = (E.real.astype(np.float32) @ Ct.real.T.astype(np.float32)
         - E.imag.astype(np.float32) @ Ct.imag.T.astype(np.float32))  # [KT, D]
    gf = g.astype(np.float32)
    Khat = K * gf[None, :]
    Khat[0] += (Dp * g).astype(np.float32)
    sigma = np.sqrt((Khat.astype(np.float64) ** 2).sum(0))
    s_d = np.maximum(QCLIP * sigma / 127.0, 1e-12).astype(np.float32)
    Kf = np.fft.rfft(Khat / s_d[None, :], n=M, axis=0)
    bf = ml_dtypes.bfloat16
    kr = np.ascontiguousarray(Kf.real.astype(bf))
    ki = np.ascontiguousarray(Kf.imag.astype(bf))
    if np.any(b):
        csK = np.cumsum(K, axis=0)
        off = b.astype(np.float32)[None, :] * (csK * gf[None, :] + (Dp * g).astype(np.float32)[None, :])
    else:
        off = None
    return kr, ki, s_d, off


def _ln_quant_bh(x, uq_cc, sr_cc, task):
    """LN + int8 quant of one (batch, L-half); writes its core shard (+ halo)."""
    bi, h2 = divmod(task, 2)
    xb = x[bi, h2 * OWN:(h2 + 1) * OWN].astype(np.float32, copy=False)
    mu = xb.mean(-1, keepdims=True)
    xc = xb - mu
    var = np.einsum("lc,lc->l", xc, xc, optimize=True)[:, None] * (1.0 / D)
    rstd = 1.0 / np.sqrt(var + LN_EPS)
    amax = np.maximum(np.abs(xc).max(-1, keepdims=True), 1e-30)
    q = np.rint(xc * (127.0 / amax)).astype(np.int8)
    s_r = (amax * rstd * (1.0 / 127.0)).astype(np.float32)[:, 0]
    c = 2 * bi + h2
    uq_cc[c * ROWS + HALO:(c + 1) * ROWS] = q
    sr_cc[c * ROWS + HALO:(c + 1) * ROWS] = s_r
    if h2 == 0:
        # halo rows for the sibling core (its preceding 512 rows)
        c1 = c + 1
        uq_cc[c1 * ROWS:c1 * ROWS + HALO] = q[OWN - HALO:]
        sr_cc[c1 * ROWS:c1 * ROWS + HALO] = s_r[OWN - HALO:]


def kernel(x, Lambda_real, Lambda_imag, C_real, C_imag, param_D, ln_gamma, ln_beta):
    import jax
    from concurrent.futures import ThreadPoolExecutor

    with _LOCK:
        if not _S.get("ready"):
            _init()

    x = np.asarray(x)
    small = [np.asarray(a) for a in (Lambda_real, Lambda_imag, C_real, C_imag,
                                     param_D, ln_gamma, ln_beta)]
    key = hashlib.sha1(b"".join(a.tobytes() for a in small)).hexdigest()
    kc = _S["kcache"]
    if key not in kc:
        kr, ki, s_d, off = _host_precompute(*small)
        kr_dev = jax.device_put(np.tile(kr, (NCORE, 1)), _S["repl"])
        ki_dev = jax.device_put(np.tile(ki, (NCORE, 1)), _S["repl"])
        kc.clear()
        kc[key] = (kr_dev, ki_dev, s_d, off)
    kr_dev, ki_dev, s_d, off = kc[key]

    uq_cc = np.zeros((NCORE * ROWS, D), np.int8)
    sr_cc = np.ones(NCORE * ROWS, np.float32)
    with ThreadPoolExecutor(NCORE) as ex:
        list(ex.map(lambda t: _ln_quant_bh(x, uq_cc, sr_cc, t), range(NCORE)))

    fc_dev, fs_dev, gr_dev, gi_dev = _S["fgdev"]
    outs = _S["sharded"](uq_cc, sr_cc, kr_dev, ki_dev, fc_dev, fs_dev,
                         gr_dev, gi_dev, _S["zeros"])
    yq = np.asarray(outs[0]).reshape(NCORE, OWN, D)

    y = np.empty((B, L, D), np.float32)
    sd_row = s_d[None, :]

    def _deq(c):
        bi, h = divmod(c, 2)
        dst = y[bi, h * OWN:(h + 1) * OWN]
        np.subtract(yq[c], np.float32(128.0), dtype=np.float32, out=dst)
        np.multiply(dst, sd_row, out=dst)

    with ThreadPoolExecutor(NCORE) as ex:
        list(ex.map(_deq, range(NCORE)))
    if off is not None:
        y[:, :KT] += off[None]
        y[:, KT:] += off[-1][None, None]
    return y



# revision 2
# speedup vs baseline: 1.4059x; 1.4059x over previous
"""DSS layer (LN -> long causal conv via overlap-save DFT matmuls -> +residual)
on 8 axon-tunneled TRN2 NeuronCores, written in Bass/Tile.

Wall-clock on this setup is dominated by the shared ~40 MB/s axon tunnel and
the single host CPU (transfers and numpy serialize), so the design minimizes
both transferred bytes and host passes:
  sharding: core = (batch b in 0..3) x (channel half h in 0..1); each core owns
        the full sequence for 512 channels -> causal conv needs NO halo rows
        (upload exactly L*D int8 = 16.8 MB, vs 21 MB with sequence sharding).
  host:  per-row mean/var of x (reductions only, no centered temp), per
        (row, half) amax of RAW x, int8 quantize q = rint(x * 127/amax).
        The LN affine is folded into per-row scale a = amax*rstd/127 and
        offset o = -mu*rstd applied on device: u = q*a + o.
  device (per core): dequant (fused scalar mul+add) -> windowed rFFT-as-matmul
        (shared F, 8 x 1024-window overlap-save, first window zero history),
        pointwise *Kf (gamma / D-residual delta tap / per-channel output scale
        folded in), inverse rFFT-as-matmul (shared G) -> uint8 quantize.
  download uint8, host dequant (s_d per channel) + beta offset (exact).

Transfers ride the single jitted shard_map call (separate device_put calls
have ~100ms per-call overhead on this tunnel); constants and the Kf spectrum
are cached on device across calls.
"""
import hashlib
import threading
import numpy as np
import ml_dtypes

B, L, D, N = 4, 4096, 1024, 512
DH = D // 2         # channels per core
CH = 512            # output chunk per window
M = 1024            # DFT window (overlap-save)
KT = 513            # kernel taps kept (<= M - CH + 1): exact for decaying K
KF = M // 2 + 1     # 513 rfft bins
NW = L // CH        # 8 windows per core
NCORE = 8
LN_EPS = 1e-5
QCLIP = 5.2
KPART = [(0, 128), (128, 128), (256, 128), (384, 128), (512, 1)]

_S = {}
_LOCK = threading.Lock()


# ---------------------------------------------------------------- device kernel
def _build_nc():
    import concourse.bacc as bacc
    import concourse.mybir as mybir
    import concourse.tile as tile

    dt = mybir.dt
    nc = bacc.Bacc("TRN2", target_bir_lowering=False, debug=False, num_devices=NCORE)
    uq_d = nc.dram_tensor("uq", [L, DH], dt.int8, kind="ExternalInput").ap()
    sa_d = nc.dram_tensor("sa", [L], dt.float32, kind="ExternalInput").ap()
    so_d = nc.dram_tensor("so", [L], dt.float32, kind="ExternalInput").ap()
    kr_d = nc.dram_tensor("kr", [KF, DH], dt.bfloat16, kind="ExternalInput").ap()
    ki_d = nc.dram_tensor("ki", [KF, DH], dt.bfloat16, kind="ExternalInput").ap()
    fc_d = nc.dram_tensor("fc", [M, KF], dt.bfloat16, kind="ExternalInput").ap()
    fs_d = nc.dram_tensor("fs", [M, KF], dt.bfloat16, kind="ExternalInput").ap()
    gr_d = nc.dram_tensor("gr", [KF, CH], dt.bfloat16, kind="ExternalInput").ap()
    gi_d = nc.dram_tensor("gi", [KF, CH], dt.bfloat16, kind="ExternalInput").ap()
    yq_d = nc.dram_tensor("yq", [L, DH], dt.uint8, kind="ExternalOutput").ap()

    nT = L // 128  # 32 row tiles of 128

    with tile.TileContext(nc) as tc:
        with (
            tc.tile_pool(name="const", bufs=1) as constp,
            tc.tile_pool(name="stage", bufs=2) as stagep,
            tc.tile_pool(name="upool", bufs=9) as upool,
            tc.tile_pool(name="uv", bufs=2) as uvp,
            tc.tile_pool(name="work", bufs=2) as workp,
            tc.tile_pool(name="psum", bufs=4, space="PSUM") as psump,
            tc.tile_pool(name="psumi", bufs=2, space="PSUM") as psumip,
        ):
            def widen(dram_ap, rows, cols, tagn):
                st = stagep.tile([rows, cols], dt.bfloat16, tag="stage")
                nc.sync.dma_start(st[:], dram_ap)
                ft = constp.tile([rows, cols], dt.float32, tag=tagn)
                nc.vector.tensor_copy(ft[:], st[:])
                return ft

            fc_t = [widen(fc_d[i * 128:(i + 1) * 128, :], 128, KF, f"fc{i}") for i in range(8)]
            fs_t = [widen(fs_d[i * 128:(i + 1) * 128, :], 128, KF, f"fs{i}") for i in range(8)]
            gr_t = [widen(gr_d[o:o + w, :], w, CH, f"gr{i}") for i, (o, w) in enumerate(KPART)]
            gi_t = [widen(gi_d[o:o + w, :], w, CH, f"gi{i}") for i, (o, w) in enumerate(KPART)]

            # Kf stays bf16 in SBUF (read by DVE pointwise; halves footprint)
            def load_bf(dram_ap, rows, cols, tagn):
                t = constp.tile([rows, cols], dt.bfloat16, tag=tagn)
                nc.sync.dma_start(t[:], dram_ap)
                return t

            kr_t = [load_bf(kr_d[o:o + w, :], w, DH, f"kr{i}") for i, (o, w) in enumerate(KPART)]
            ki_t = [load_bf(ki_d[o:o + w, :], w, DH, f"ki{i}") for i, (o, w) in enumerate(KPART)]

            sa_raw = constp.tile([128, nT], dt.float32, tag="sa_raw")
            nc.sync.dma_start(sa_raw[:], sa_d.rearrange("(n p) -> p n", p=128))
            so_raw = constp.tile([128, nT], dt.float32, tag="so_raw")
            nc.sync.dma_start(so_raw[:], so_d.rearrange("(n p) -> p n", p=128))
            # staged via same-engine copy so dequant TensorScalarPtr needs no waits
            sa_sb = constp.tile([128, nT], dt.float32, tag="sa_sb")
            nc.vector.tensor_copy(sa_sb[:], sa_raw[:])
            so_sb = constp.tile([128, nT], dt.float32, tag="so_sb")
            nc.vector.tensor_copy(so_sb[:], so_raw[:])

            mult = mybir.AluOpType.mult
            add = mybir.AluOpType.add

            for w in range(NW):
                # window rows [w*512-512, w*512+512); slot si holds global tile
                # gt = w*4 - 4 + si; for w == 0 slots 0-3 are zero history and
                # their matmuls are skipped entirely.
                s0 = 4 if w == 0 else 0
                u_t = [None] * 8
                for si in range(s0, 8):
                    gt = w * 4 - 4 + si
                    stq = stagep.tile([128, DH], dt.int8, tag="uqstage")
                    nc.sync.dma_start(stq[:], uq_d[gt * 128:(gt + 1) * 128, :])
                    uf = upool.tile([128, DH], dt.float32, tag="u")
                    nc.vector.tensor_scalar(
                        uf[:], stq[:], sa_sb[:, gt:gt + 1], so_sb[:, gt:gt + 1],
                        mult, add,
                    )
                    u_t[si] = uf
                Vr, Vi = [], []
                for it, (ko, kw) in enumerate(KPART):
                    sb_ri = []
                    for nm, fT in (("ur", fc_t), ("ui", fs_t)):
                        ps = psump.tile([kw, DH], dt.float32, tag="psf")
                        for si in range(s0, 8):
                            nc.tensor.matmul(
                                ps[:], fT[si][:, ko:ko + kw], u_t[si][:],
                                start=(si == s0), stop=(si == 7),
                            )
                        sb = uvp.tile([kw, DH], dt.float32, tag=nm)
                        nc.scalar.copy(sb[:], ps[:])
                        sb_ri.append(sb)
                    ur, ui = sb_ri
                    krs, kis = kr_t[it][:kw, :], ki_t[it][:kw, :]
                    t1 = workp.tile([kw, DH], dt.float32, tag="t1")
                    t2 = workp.tile([kw, DH], dt.float32, tag="t2")
                    nc.vector.tensor_mul(t1[:], ur[:], krs)
                    nc.vector.tensor_mul(t2[:], ui[:], kis)
                    vr = uvp.tile([kw, DH], dt.float32, tag=f"vr{it}")
                    nc.vector.tensor_sub(vr[:], t1[:], t2[:])
                    t3 = workp.tile([kw, DH], dt.float32, tag="t3")
                    t4 = workp.tile([kw, DH], dt.float32, tag="t4")
                    nc.vector.tensor_mul(t3[:], ur[:], kis)
                    nc.vector.tensor_mul(t4[:], ui[:], krs)
                    vi = uvp.tile([kw, DH], dt.float32, tag=f"vi{it}")
                    nc.vector.tensor_add(vi[:], t3[:], t4[:])
                    Vr.append(vr)
                    Vi.append(vi)
                for tt in range(4):
                    ps = psumip.tile([128, DH], dt.float32, tag="psi")
                    mm = 0
                    for gT, V in ((gr_t, Vr), (gi_t, Vi)):
                        for it, (ko, kw) in enumerate(KPART):
                            nc.tensor.matmul(
                                ps[:], gT[it][:kw, tt * 128:(tt + 1) * 128], V[it][:],
                                start=(mm == 0), stop=(mm == 9),
                            )
                            mm += 1
                    yf = workp.tile([128, DH], dt.float32, tag="yf")
                    nc.vector.tensor_scalar(yf[:], ps[:], 128.0, 1.0, add,
                                            mybir.AluOpType.max)
                    nc.vector.tensor_scalar_min(yf[:], yf[:], 255.0)
                    yq_t = workp.tile([128, DH], dt.uint8, tag="yqt")
                    nc.vector.tensor_copy(yq_t[:], yf[:])
                    nc.sync.dma_start(
                        yq_d[w * CH + tt * 128: w * CH + (tt + 1) * 128, :], yq_t[:]
                    )
    nc.finalize()
    return nc


# ---------------------------------------------------------------- runner
def _make_runner(nc):
    import jax
    from jax.sharding import Mesh, PartitionSpec
    from jax.experimental.shard_map import shard_map
    import concourse.mybir as mybir
    from concourse.bass2jax import install_neuronx_cc_hook, _bass_exec_p, partition_id_tensor

    install_neuronx_cc_hook()
    in_names, out_names, out_avals, zero_outs = [], [], [], []
    partition_name = nc.partition_id_tensor.name if nc.partition_id_tensor else None
    for alloc in nc.m.functions[0].allocations:
        if not isinstance(alloc, mybir.MemoryLocationSet):
            continue
        name = alloc.memorylocations[0].name
        if alloc.kind == "ExternalInput":
            if name != partition_name:
                in_names.append(name)
        elif alloc.kind == "ExternalOutput":
            out_names.append(name)
            shape = tuple(alloc.tensor_shape)
            dtype = mybir.dt.np(alloc.dtype)
            out_avals.append(jax.core.ShapedArray(shape, dtype))
            zero_outs.append(np.zeros(shape, dtype))
    n_params = len(in_names)
    all_names = in_names + out_names
    if partition_name is not None:
        all_names.append(partition_name)

    def _body(*args):
        operands = list(args)
        if partition_name is not None:
            operands.append(partition_id_tensor())
        outs = _bass_exec_p.bind(
            *operands,
            out_avals=tuple(out_avals),
            in_names=tuple(all_names),
            out_names=tuple(out_names),
            lowering_input_output_aliases=(),
            sim_require_finite=True,
            sim_require_nnan=True,
            nc=nc,
        )
        return tuple(outs)

    devices = jax.devices()[:NCORE]
    mesh = Mesh(np.asarray(devices), ("core",))
    n_outs = len(out_names)
    sharded = jax.jit(
        shard_map(
            _body, mesh=mesh,
            in_specs=(PartitionSpec("core"),) * (n_params + n_outs),
            out_specs=(PartitionSpec("core"),) * n_outs,
            check_rep=False,
        ),
        keep_unused=True,
    )
    return sharded, in_names, out_names, zero_outs, mesh


def _dft_consts():
    t = np.arange(M)
    k = np.arange(KF)
    ang = 2.0 * np.pi / M * np.outer(t, k)
    fc = np.cos(ang)
    fs = -np.sin(ang)
    w_k = np.where((k == 0) | (k == M // 2), 1.0, 2.0) / M
    angi = 2.0 * np.pi / M * np.outer(k, np.arange(CH, M))
    gr = w_k[:, None] * np.cos(angi)
    gi = -w_k[:, None] * np.sin(angi)
    bf = ml_dtypes.bfloat16
    return (fc.astype(bf), fs.astype(bf), gr.astype(bf), gi.astype(bf))


def _init():
    import jax
    from jax.sharding import NamedSharding, PartitionSpec

    nc = _build_nc()
    sharded, in_names, out_names, zero_outs, mesh = _make_runner(nc)
    assert in_names == ["uq", "sa", "so", "kr", "ki", "fc", "fs", "gr", "gi"], in_names
    repl = NamedSharding(mesh, PartitionSpec("core"))
    fc, fs, gr, gi = _dft_consts()
    tile8 = lambda a: jax.device_put(np.tile(a, (NCORE, 1)), repl)
    _S["fgdev"] = (tile8(fc), tile8(fs), tile8(gr), tile8(gi))
    zc = np.zeros((NCORE * L, DH), np.uint8)
    _S["zeros"] = jax.device_put(zc, repl)
    _S["sharded"] = sharded
    _S["repl"] = repl
    _S["kcache"] = {}
    _S["ready"] = True


# ---------------------------------------------------------------- host math
def _host_precompute(Lr, Li, Cr, Ci, Dp, g, b):
    lam = -np.exp(Lr.astype(np.float64)) + 1j * np.exp(Li.astype(np.float64))
    Ct = (Cr.astype(np.float64) + 1j * Ci.astype(np.float64)) * (np.exp(lam) - 1.0) / lam
    tau = np.arange(KT)
    E = np.exp(lam[None, :] * tau[:, None])  # [KT, N]
    K = (E.real.astype(np.float32) @ Ct.real.T.astype(np.float32)
         - E.imag.astype(np.float32) @ Ct.imag.T.astype(np.float32))  # [KT, D]
    gf = g.astype(np.float32)
    Khat = K * gf[None, :]
    Khat[0] += (Dp * g).astype(np.float32)
    sigma = np.sqrt((Khat.astype(np.float64) ** 2).sum(0))
    s_d = np.maximum(QCLIP * sigma / 127.0, 1e-12).astype(np.float32)
    Kf = np.fft.rfft(Khat / s_d[None, :], n=M, axis=0)
    bf = ml_dtypes.bfloat16
    kr = Kf.real.astype(bf)
    ki = Kf.imag.astype(bf)
    # per-core d-half slices, stacked core-major for the sharded input
    krs = np.ascontiguousarray(
        np.concatenate([kr[:, (c % 2) * DH:(c % 2 + 1) * DH] for c in range(NCORE)], 0))
    kis = np.ascontiguousarray(
        np.concatenate([ki[:, (c % 2) * DH:(c % 2 + 1) * DH] for c in range(NCORE)], 0))
    if np.any(b):
        csK = np.cumsum(K, axis=0)
        off = b.astype(np.float32)[None, :] * (csK * gf[None, :] + (Dp * g).astype(np.float32)[None, :])
    else:
        off = None
    return krs, kis, s_d, off


def kernel(x, Lambda_real, Lambda_imag, C_real, C_imag, param_D, ln_gamma, ln_beta):
    import jax

    with _LOCK:
        if not _S.get("ready"):
            _init()

    x = np.asarray(x)
    small = [np.asarray(a) for a in (Lambda_real, Lambda_imag, C_real, C_imag,
                                     param_D, ln_gamma, ln_beta)]
    key = hashlib.sha1(b"".join(a.tobytes() for a in small)).hexdigest()
    kc = _S["kcache"]
    if key not in kc:
        krs, kis, s_d, off = _host_precompute(*small)
        kr_dev = jax.device_put(krs, _S["repl"])
        ki_dev = jax.device_put(kis, _S["repl"])
        kc.clear()
        kc[key] = (kr_dev, ki_dev, s_d, off)
    kr_dev, ki_dev, s_d, off = kc[key]

    # ---- LN stats + raw-x int8 quantization (single CPU: plain serial numpy)
    uq_cc = np.empty((NCORE * L, DH), np.int8)
    sa_cc = np.empty((NCORE, L), np.float32)
    so_cc = np.empty((NCORE, L), np.float32)
    tf = np.empty((L, DH), np.float32)
    for bi in range(B):
        xb = x[bi]
        mu = xb.mean(1)
        sq = np.einsum('lc,lc->l', xb, xb, optimize=True)
        var = sq * (1.0 / D) - mu * mu
        rstd = 1.0 / np.sqrt(var + LN_EPS)
        for h in range(2):
            c = 2 * bi + h
            xh = xb[:, h * DH:(h + 1) * DH]
            amax = np.maximum(xh.max(1), -xh.min(1))
            np.maximum(amax, 1e-30, out=amax)
            s_q = np.float32(127.0) / amax
            np.multiply(xh, s_q[:, None], out=tf)
            np.rint(tf, out=tf)
            np.copyto(uq_cc[c * L:(c + 1) * L], tf, casting='unsafe')
            sa_cc[c] = amax * rstd * np.float32(1.0 / 127.0)
            so_cc[c] = -mu * rstd

    fc_dev, fs_dev, gr_dev, gi_dev = _S["fgdev"]
    outs = _S["sharded"](uq_cc, sa_cc.reshape(-1), so_cc.reshape(-1),
                         kr_dev, ki_dev, fc_dev, fs_dev, gr_dev, gi_dev,
                         _S["zeros"])
    yq = np.asarray(outs[0]).reshape(NCORE, L, DH)

    # ---- dequant: y = (yq - 128) * s_d  (2 passes, strided half writes)
    y = np.empty((B, L, D), np.float32)
    for c in range(NCORE):
        bi, h = divmod(c, 2)
        sd_h = s_d[h * DH:(h + 1) * DH]
        dst = y[bi, :, h * DH:(h + 1) * DH]
        np.multiply(yq[c], sd_h[None, :], out=dst)
        np.subtract(dst, (np.float32(128.0) * sd_h)[None, :], out=dst)
    if off is not None:
        y[:, :KT] += off[None]
        y[:, KT:] += off[-1][None, None]
    return y


# revision 3
# speedup vs baseline: 1.5731x; 1.1189x over previous
"""DSS layer (LN -> long causal conv via overlap-save DFT matmuls -> +residual)
on 8 axon-tunneled TRN2 NeuronCores, written in Bass/Tile.

Wall-clock on this setup is dominated by the shared ~40 MB/s axon tunnel and
the single host CPU (transfers and numpy serialize), so the design minimizes
transferred bytes, host passes, and serialized round-trip latency:
  sharding: one program where core = (batch in a pair) x (channel quarter);
        the program is dispatched twice (batches 0-1, then 2-3). Each core owns
        the full sequence for 256 channels -> causal conv needs NO halo rows
        (total upload exactly L*D int8 = 16.8 MB), and the two async dispatches
        let each exec's ~50-90 ms round-trip hide under the other call's
        CPU-bound transfer work.
  host:  per-row mean/var of x (reductions only, no centered temp), per
        (row, quarter) amax of RAW x, int8 quantize q = rint(x * 127/amax).
        The LN affine is folded into per-row scale a = amax*rstd/127 and
        offset o = -mu*rstd applied on device: u = q*a + o.
  device (per core): dequant (fused scalar mul+add) -> windowed rFFT-as-matmul
        (shared F, 8 x 1024-window overlap-save, first window zero history),
        pointwise *Kf (gamma / D-residual delta tap / per-channel output scale
        folded in), inverse rFFT-as-matmul (shared G) -> uint8 quantize.
  download uint8 per call, host dequant (s_d per channel) + beta offset.

Transfers ride the jitted shard_map dispatches (separate device_put calls have
~60-130 ms per-call overhead on this tunnel); constants and the Kf spectrum
are cached on device across calls.
"""
import hashlib
import threading
import numpy as np
import ml_dtypes

B, L, D, N = 4, 4096, 1024, 512
NQ = 4              # channel quarters
DH = D // NQ        # 256 channels per core
CH = 512            # output chunk per window
M = 1024            # DFT window (overlap-save)
KT = 513            # kernel taps kept (<= M - CH + 1): exact for decaying K
KF = M // 2 + 1     # 513 rfft bins
NW = L // CH        # 8 windows per core
NCORE = 8
BPC = 2             # batches per call
LN_EPS = 1e-5
QCLIP = 5.2
KPART = [(0, 128), (128, 128), (256, 128), (384, 128), (512, 1)]

_S = {}
_LOCK = threading.Lock()


# ---------------------------------------------------------------- device kernel
def _build_nc():
    import concourse.bacc as bacc
    import concourse.mybir as mybir
    import concourse.tile as tile

    dt = mybir.dt
    nc = bacc.Bacc("TRN2", target_bir_lowering=False, debug=False, num_devices=NCORE)
    uq_d = nc.dram_tensor("uq", [L, DH], dt.int8, kind="ExternalInput").ap()
    sa_d = nc.dram_tensor("sa", [L], dt.float32, kind="ExternalInput").ap()
    so_d = nc.dram_tensor("so", [L], dt.float32, kind="ExternalInput").ap()
    kr_d = nc.dram_tensor("kr", [KF, DH], dt.bfloat16, kind="ExternalInput").ap()
    ki_d = nc.dram_tensor("ki", [KF, DH], dt.bfloat16, kind="ExternalInput").ap()
    fc_d = nc.dram_tensor("fc", [M, KF], dt.bfloat16, kind="ExternalInput").ap()
    fs_d = nc.dram_tensor("fs", [M, KF], dt.bfloat16, kind="ExternalInput").ap()
    gr_d = nc.dram_tensor("gr", [KF, CH], dt.bfloat16, kind="ExternalInput").ap()
    gi_d = nc.dram_tensor("gi", [KF, CH], dt.bfloat16, kind="ExternalInput").ap()
    yq_d = nc.dram_tensor("yq", [L, DH], dt.uint8, kind="ExternalOutput").ap()

    nT = L // 128  # 32 row tiles of 128

    with tile.TileContext(nc) as tc:
        with (
            tc.tile_pool(name="const", bufs=1) as constp,
            tc.tile_pool(name="stage", bufs=2) as stagep,
            tc.tile_pool(name="upool", bufs=9) as upool,
            tc.tile_pool(name="uv", bufs=2) as uvp,
            tc.tile_pool(name="work", bufs=2) as workp,
            tc.tile_pool(name="psum", bufs=4, space="PSUM") as psump,
            tc.tile_pool(name="psumi", bufs=2, space="PSUM") as psumip,
        ):
            def widen(dram_ap, rows, cols, tagn):
                st = stagep.tile([rows, cols], dt.bfloat16, tag="stage")
                nc.sync.dma_start(st[:], dram_ap)
                ft = constp.tile([rows, cols], dt.float32, tag=tagn)
                nc.vector.tensor_copy(ft[:], st[:])
                return ft

            fc_t = [widen(fc_d[i * 128:(i + 1) * 128, :], 128, KF, f"fc{i}") for i in range(8)]
            fs_t = [widen(fs_d[i * 128:(i + 1) * 128, :], 128, KF, f"fs{i}") for i in range(8)]
            gr_t = [widen(gr_d[o:o + w, :], w, CH, f"gr{i}") for i, (o, w) in enumerate(KPART)]
            gi_t = [widen(gi_d[o:o + w, :], w, CH, f"gi{i}") for i, (o, w) in enumerate(KPART)]

            # Kf stays bf16 in SBUF (read by DVE pointwise; halves footprint)
            def load_bf(dram_ap, rows, cols, tagn):
                t = constp.tile([rows, cols], dt.bfloat16, tag=tagn)
                nc.sync.dma_start(t[:], dram_ap)
                return t

            kr_t = [load_bf(kr_d[o:o + w, :], w, DH, f"kr{i}") for i, (o, w) in enumerate(KPART)]
            ki_t = [load_bf(ki_d[o:o + w, :], w, DH, f"ki{i}") for i, (o, w) in enumerate(KPART)]

            sa_raw = constp.tile([128, nT], dt.float32, tag="sa_raw")
            nc.sync.dma_start(sa_raw[:], sa_d.rearrange("(n p) -> p n", p=128))
            so_raw = constp.tile([128, nT], dt.float32, tag="so_raw")
            nc.sync.dma_start(so_raw[:], so_d.rearrange("(n p) -> p n", p=128))
            # staged via same-engine copy so dequant TensorScalarPtr needs no waits
            sa_sb = constp.tile([128, nT], dt.float32, tag="sa_sb")
            nc.vector.tensor_copy(sa_sb[:], sa_raw[:])
            so_sb = constp.tile([128, nT], dt.float32, tag="so_sb")
            nc.vector.tensor_copy(so_sb[:], so_raw[:])

            mult = mybir.AluOpType.mult
            add = mybir.AluOpType.add

            for w in range(NW):
                # window rows [w*512-512, w*512+512); slot si holds global tile
                # gt = w*4 - 4 + si; for w == 0 slots 0-3 are zero history and
                # their matmuls are skipped entirely.
                s0 = 4 if w == 0 else 0
                u_t = [None] * 8
                for si in range(s0, 8):
                    gt = w * 4 - 4 + si
                    stq = stagep.tile([128, DH], dt.int8, tag="uqstage")
                    nc.sync.dma_start(stq[:], uq_d[gt * 128:(gt + 1) * 128, :])
                    uf = upool.tile([128, DH], dt.float32, tag="u")
                    nc.vector.tensor_scalar(
                        uf[:], stq[:], sa_sb[:, gt:gt + 1], so_sb[:, gt:gt + 1],
                        mult, add,
                    )
                    u_t[si] = uf
                Vr, Vi = [], []
                for it, (ko, kw) in enumerate(KPART):
                    sb_ri = []
                    for nm, fT in (("ur", fc_t), ("ui", fs_t)):
                        ps = psump.tile([kw, DH], dt.float32, tag="psf")
                        for si in range(s0, 8):
                            nc.tensor.matmul(
                                ps[:], fT[si][:, ko:ko + kw], u_t[si][:],
                                start=(si == s0), stop=(si == 7),
                            )
                        sb = uvp.tile([kw, DH], dt.float32, tag=nm)
                        nc.scalar.copy(sb[:], ps[:])
                        sb_ri.append(sb)
                    ur, ui = sb_ri
                    krs, kis = kr_t[it][:kw, :], ki_t[it][:kw, :]
                    t1 = workp.tile([kw, DH], dt.float32, tag="t1")
                    t2 = workp.tile([kw, DH], dt.float32, tag="t2")
                    nc.vector.tensor_mul(t1[:], ur[:], krs)
                    nc.vector.tensor_mul(t2[:], ui[:], kis)
                    vr = uvp.tile([kw, DH], dt.float32, tag=f"vr{it}")
                    nc.vector.tensor_sub(vr[:], t1[:], t2[:])
                    t3 = workp.tile([kw, DH], dt.float32, tag="t3")
                    t4 = workp.tile([kw, DH], dt.float32, tag="t4")
                    nc.vector.tensor_mul(t3[:], ur[:], kis)
                    nc.vector.tensor_mul(t4[:], ui[:], krs)
                    vi = uvp.tile([kw, DH], dt.float32, tag=f"vi{it}")
                    nc.vector.tensor_add(vi[:], t3[:], t4[:])
                    Vr.append(vr)
                    Vi.append(vi)
                for tt in range(4):
                    ps = psumip.tile([128, DH], dt.float32, tag="psi")
                    mm = 0
                    for gT, V in ((gr_t, Vr), (gi_t, Vi)):
                        for it, (ko, kw) in enumerate(KPART):
                            nc.tensor.matmul(
                                ps[:], gT[it][:kw, tt * 128:(tt + 1) * 128], V[it][:],
                                start=(mm == 0), stop=(mm == 9),
                            )
                            mm += 1
                    yf = workp.tile([128, DH], dt.float32, tag="yf")
                    nc.vector.tensor_scalar(yf[:], ps[:], 128.0, 1.0, add,
                                            mybir.AluOpType.max)
                    nc.vector.tensor_scalar_min(yf[:], yf[:], 255.0)
                    yq_t = workp.tile([128, DH], dt.uint8, tag="yqt")
                    nc.vector.tensor_copy(yq_t[:], yf[:])
                    nc.sync.dma_start(
                        yq_d[w * CH + tt * 128: w * CH + (tt + 1) * 128, :], yq_t[:]
                    )
    nc.finalize()
    return nc


# ---------------------------------------------------------------- runner
def _make_runner(nc):
    import jax
    from jax.sharding import Mesh, PartitionSpec
    from jax.experimental.shard_map import shard_map
    import concourse.mybir as mybir
    from concourse.bass2jax import install_neuronx_cc_hook, _bass_exec_p, partition_id_tensor

    install_neuronx_cc_hook()
    in_names, out_names, out_avals, zero_outs = [], [], [], []
    partition_name = nc.partition_id_tensor.name if nc.partition_id_tensor else None
    for alloc in nc.m.functions[0].allocations:
        if not isinstance(alloc, mybir.MemoryLocationSet):
            continue
        name = alloc.memorylocations[0].name
        if alloc.kind == "ExternalInput":
            if name != partition_name:
                in_names.append(name)
        elif alloc.kind == "ExternalOutput":
            out_names.append(name)
            shape = tuple(alloc.tensor_shape)
            dtype = mybir.dt.np(alloc.dtype)
            out_avals.append(jax.core.ShapedArray(shape, dtype))
            zero_outs.append(np.zeros(shape, dtype))
    n_params = len(in_names)
    all_names = in_names + out_names
    if partition_name is not None:
        all_names.append(partition_name)

    def _body(*args):
        operands = list(args)
        if partition_name is not None:
            operands.append(partition_id_tensor())
        outs = _bass_exec_p.bind(
            *operands,
            out_avals=tuple(out_avals),
            in_names=tuple(all_names),
            out_names=tuple(out_names),
            lowering_input_output_aliases=(),
            sim_require_finite=True,
            sim_require_nnan=True,
            nc=nc,
        )
        return tuple(outs)

    devices = jax.devices()[:NCORE]
    mesh = Mesh(np.asarray(devices), ("core",))
    n_outs = len(out_names)
    sharded = jax.jit(
        shard_map(
            _body, mesh=mesh,
            in_specs=(PartitionSpec("core"),) * (n_params + n_outs),
            out_specs=(PartitionSpec("core"),) * n_outs,
            check_rep=False,
        ),
        keep_unused=True,
    )
    return sharded, in_names, out_names, zero_outs, mesh


def _dft_consts():
    t = np.arange(M)
    k = np.arange(KF)
    ang = 2.0 * np.pi / M * np.outer(t, k)
    fc = np.cos(ang)
    fs = -np.sin(ang)
    w_k = np.where((k == 0) | (k == M // 2), 1.0, 2.0) / M
    angi = 2.0 * np.pi / M * np.outer(k, np.arange(CH, M))
    gr = w_k[:, None] * np.cos(angi)
    gi = -w_k[:, None] * np.sin(angi)
    bf = ml_dtypes.bfloat16
    return (fc.astype(bf), fs.astype(bf), gr.astype(bf), gi.astype(bf))


def _init():
    import jax
    from jax.sharding import NamedSharding, PartitionSpec

    nc = _build_nc()
    sharded, in_names, out_names, zero_outs, mesh = _make_runner(nc)
    assert in_names == ["uq", "sa", "so", "kr", "ki", "fc", "fs", "gr", "gi"], in_names
    repl = NamedSharding(mesh, PartitionSpec("core"))
    fc, fs, gr, gi = _dft_consts()
    tile8 = lambda a: jax.device_put(np.tile(a, (NCORE, 1)), repl)
    _S["fgdev"] = (tile8(fc), tile8(fs), tile8(gr), tile8(gi))
    zc = np.zeros((NCORE * L, DH), np.uint8)
    _S["zeros"] = jax.device_put(zc, repl)
    _S["sharded"] = sharded
    _S["repl"] = repl
    _S["kcache"] = {}
    _S["ready"] = True


# ---------------------------------------------------------------- host math
def _host_precompute(Lr, Li, Cr, Ci, Dp, g, b):
    lam = -np.exp(Lr.astype(np.float64)) + 1j * np.exp(Li.astype(np.float64))
    Ct = (Cr.astype(np.float64) + 1j * Ci.astype(np.float64)) * (np.exp(lam) - 1.0) / lam
    tau = np.arange(KT)
    E = np.exp(lam[None, :] * tau[:, None])  # [KT, N]
    K = (E.real.astype(np.float32) @ Ct.real.T.astype(np.float32)
         - E.imag.astype(np.float32) @ Ct.imag.T.astype(np.float32))  # [KT, D]
    gf = g.astype(np.float32)
    Khat = K * gf[None, :]
    Khat[0] += (Dp * g).astype(np.float32)
    sigma = np.sqrt((Khat.astype(np.float64) ** 2).sum(0))
    s_d = np.maximum(QCLIP * sigma / 127.0, 1e-12).astype(np.float32)
    Kf = np.fft.rfft(Khat / s_d[None, :], n=M, axis=0)
    bf = ml_dtypes.bfloat16
    kr = Kf.real.astype(bf)
    ki = Kf.imag.astype(bf)
    # per-core d-quarter slices, stacked core-major for the sharded input
    krs = np.ascontiguousarray(
        np.concatenate([kr[:, (c % NQ) * DH:(c % NQ + 1) * DH] for c in range(NCORE)], 0))
    kis = np.ascontiguousarray(
        np.concatenate([ki[:, (c % NQ) * DH:(c % NQ + 1) * DH] for c in range(NCORE)], 0))
    if np.any(b):
        csK = np.cumsum(K, axis=0)
        off = b.astype(np.float32)[None, :] * (csK * gf[None, :] + (Dp * g).astype(np.float32)[None, :])
    else:
        off = None
    return krs, kis, s_d, off


def _quant_pair(x, p, uq_cc, sa_cc, so_cc, tf):
    """LN stats + raw-x int8 quantization for batch pair p (batches 2p, 2p+1).

    Core c of the call handles batch 2p + c//NQ, channel quarter c%NQ.
    """
    for bl in range(BPC):
        bi = BPC * p + bl
        xb = x[bi]
        mu = xb.mean(1)
        sq = np.einsum('lc,lc->l', xb, xb, optimize=True)
        var = sq * (1.0 / D) - mu * mu
        rstd = 1.0 / np.sqrt(var + LN_EPS)
        nmu = -mu * rstd
        for q in range(NQ):
            c = NQ * bl + q
            xh = xb[:, q * DH:(q + 1) * DH]
            amax = np.maximum(xh.max(1), -xh.min(1))
            np.maximum(amax, 1e-30, out=amax)
            s_q = np.float32(127.0) / amax
            np.multiply(xh, s_q[:, None], out=tf)
            np.rint(tf, out=tf)
            np.copyto(uq_cc[c * L:(c + 1) * L], tf, casting='unsafe')
            sa_cc[c] = amax * rstd * np.float32(1.0 / 127.0)
            so_cc[c] = nmu


def kernel(x, Lambda_real, Lambda_imag, C_real, C_imag, param_D, ln_gamma, ln_beta):
    with _LOCK:
        if not _S.get("ready"):
            _init()

    x = np.asarray(x)
    small = [np.asarray(a) for a in (Lambda_real, Lambda_imag, C_real, C_imag,
                                     param_D, ln_gamma, ln_beta)]
    key = hashlib.sha1(b"".join(a.tobytes() for a in small)).hexdigest()
    kc = _S["kcache"]
    if key not in kc:
        import jax
        krs, kis, s_d, off = _host_precompute(*small)
        kr_dev = jax.device_put(krs, _S["repl"])
        ki_dev = jax.device_put(kis, _S["repl"])
        kc.clear()
        kc[key] = (kr_dev, ki_dev, s_d, off)
    kr_dev, ki_dev, s_d, off = kc[key]
    fc_dev, fs_dev, gr_dev, gi_dev = _S["fgdev"]
    sharded, zeros = _S["sharded"], _S["zeros"]

    # ---- per batch-pair: quantize then dispatch (async); execs overlap the
    #      other call's CPU-bound transfer work.
    tf = np.empty((L, DH), np.float32)
    uq = [np.empty((NCORE * L, DH), np.int8) for _ in range(2)]
    sa = [np.empty((NCORE, L), np.float32) for _ in range(2)]
    so = [np.empty((NCORE, L), np.float32) for _ in range(2)]
    outs = [None, None]
    for p in range(2):
        _quant_pair(x, p, uq[p], sa[p], so[p], tf)
        outs[p] = sharded(uq[p], sa[p].reshape(-1), so[p].reshape(-1),
                          kr_dev, ki_dev, fc_dev, fs_dev, gr_dev, gi_dev,
                          zeros)

    # ---- fetch + dequant per call: y = (yq - 128) * s_d
    y = np.empty((B, L, D), np.float32)
    for p in range(2):
        yq = np.asarray(outs[p][0]).reshape(NCORE, L, DH)
        for c in range(NCORE):
            bi, q = BPC * p + c // NQ, c % NQ
            sd_q = s_d[q * DH:(q + 1) * DH]
            dst = y[bi, :, q * DH:(q + 1) * DH]
            np.multiply(yq[c], sd_q[None, :], out=dst)
            np.subtract(dst, (np.float32(128.0) * sd_q)[None, :], out=dst)
    if off is not None:
        y[:, :KT] += off[None]
        y[:, KT:] += off[-1][None, None]
    return y
